# revision 68
# baseline (speedup 1.0000x reference)
"""AgreementRouting (CapsNet dynamic routing) Trainium2 kernel.

Data-parallel over batch B=128 across 8 cores (B_local=16 per core).

Per core, u lives in SBUF twice, as fp16:
  u16: partition p = b_loc*16 + d   (b_loc in [0,8), d in [0,16))
       free       = (j in [0,10), h in [0,2), i in [0,1152))
  uT:  partition  = i_lo = i % 128
       free       = (h, j, ci = i//128 in [0,9), p = (b_loc, d))
local batch index beta = h*8 + b_loc.

Structure per routing iteration (one "phase" per (iteration, h); the
two h-streams are phase-shifted by half an iteration so each stream's
cross-engine softmax/W4 tail latency hides under the other stream's W1
matmuls; the previous phase's W4 matmuls are emitted mid-W1 so its
extract chain completes during the current W1):
  W1: PE accumulating matmuls a_c = sum_j sfat[j].T @ u16[j]    (fp16)
      (sfat_all = 10 block-diagonal stationaries packed at 88-col
      stride, rebuilt from s in ONE strided DVE tensor_tensor)
  bb += f80 * a_c                   (DVE STT, squash scale f
                                     folded into the logit update)
  softmax in i-major layout: PE-transpose bb 128-col chunks into
      bbT [i_lo, (ci, j, b)] f32 PSUM, exp on ACT, Z = sum over j via
      strided DVE reduce, c16 = e * recip(Z) with a stride-0 broadcast
      (big pieces on GpSimd) -> cT16 [i_lo, (ci, j, b)] fp16 directly
      in the transposed layout
  W4: weighted sum on PE with uT as the *stationary*:
      s_ps[(b,d), (j,c,b')] += sum_i uT[i,(b,d)] * cT16[i,(j,b')]
      per 3 i-pieces, then one masked DVE reduce extracts the b'==b
      diagonal into s.
  squash scale f80 computed entirely in (j,b)-partition layout
      (PE matmul + masked STT), no SBUF-shuffle DMAs; the final
      v = f*s replicates f to [(b,d), j] via a jmask scale + b82aT
      matmul.

Numerics vs the fp32 oracle: absmax/scale ~ 7e-4.
"""

import os
import sys

import numpy as np

for _p in ("/opt/trn_rl_repo", "/opt/trn_rl_repo/concourse"):
    if _p not in sys.path and os.path.isdir(_p):
        sys.path.insert(0, _p)

B, IC, OC, D = 128, 1152, 10, 16
NCORES = 8
BL = B // NCORES          # 16 local batch
H = 2                     # halves of local batch
BLOC = BL // H            # 8
NI = IC                   # 1152
NC9 = NI // 128           # 9 i-chunks of 128
EPS = 1e-8
NITER = 3
CHUNKS = [(0, 512), (512, 1024), (1024, 1152)]
TIME_REPS = int(os.environ.get("K_TIME_REPS", "1"))  # whole-program reps

_PROG_CACHE = {}


def _build_consts():
    """Host-side constant selector/mask matrices."""
    # base8[(b,d), b2] = 1 if b==b2                      -> [128, 8] f32
    base8 = np.zeros((BLOC * D, BLOC), np.float32)
    for b in range(BLOC):
        base8[b * D:(b + 1) * D, b] = 1.0
    # b82a[(b,d), (j,b2)] = 1 if b==b2                   -> [128, 80]
    b82a = np.tile(base8, (1, OC)).astype(np.float32)
    # b82a3: same mask tiled over (j, piece, b2)         -> [128, 240]
    b82a3 = np.tile(base8, (1, OC * 3)).astype(np.float32)
    # jmask[(j,b), j2] = 1 if j==j2                      -> [80, 10] f32
    jmask = np.zeros((OC * BLOC, OC), np.float32)
    for j in range(OC):
        jmask[j * BLOC:(j + 1) * BLOC, j] = 1.0
    ident80 = np.eye(OC * BLOC, dtype=np.float32)
    # pack all consts into two [128, *] arrays (one DMA each); the
    # 80-partition tensors sit in rows 0..79 with zero padding below.
    f32pack = np.zeros((128, 8 + 240 + 10 + 80), np.float32)
    f32pack[:, 0:8] = base8
    f32pack[:, 8:248] = b82a3
    f32pack[:80, 248:258] = jmask
    f32pack[:80, 258:338] = ident80
    f16pack = np.zeros((128, 80 + 128), np.float16)
    f16pack[:, 0:80] = b82a.astype(np.float16)
    f16pack[:80, 80:208] = b82a.T.astype(np.float16)
    return dict(f32pack=f32pack, f16pack=f16pack)


def _build_program(general_b):
    import concourse.bacc as bacc
    import concourse.mybir as mybir
    import concourse.tile as tile

    dt = mybir.dt
    AF = mybir.ActivationFunctionType
    ALU = mybir.AluOpType
    AX = mybir.AxisListType

    # Force a single shared ACT table (Exp+Ln+Copy+Identity in one set) so
    # the table-load pass emits one load instead of thrashing per func.
    from concourse import hw_specs as _hws
    _orig_tabs = _hws.get_activation_tables
    _keep = "natural_log_exp_and_others"

    def _patched_tabs(arch, __orig=_orig_tabs, __keep=_keep):
        tabs = __orig(arch)
        return {n: (s if n == __keep else set()) for n, s in tabs.items()}

    bacc.get_activation_tables = _patched_tabs

    nc = bacc.Bacc("TRN2", target_bir_lowering=False, debug=False)

    # ---- DRAM I/O ----
    u16_d = nc.dram_tensor("u16", [128, OC, H, NI], dt.float16,
                           kind="ExternalInput").ap()
    uT_d = nc.dram_tensor("uT", [128, H, OC, NC9 * 128], dt.float16,
                          kind="ExternalInput").ap()
    f32pack_d = nc.dram_tensor("f32pack", [128, 338], dt.float32,
                               kind="ExternalInput").ap()
    f16pack_d = nc.dram_tensor("f16pack", [128, 208], dt.float16,
                               kind="ExternalInput").ap()
    if general_b:
        c0_d = nc.dram_tensor("c0rep", [128, OC, NI], dt.float16,
                              kind="ExternalInput").ap()
        bb0_d = nc.dram_tensor("bb0", [OC * BLOC, NI], dt.float32,
                               kind="ExternalInput").ap()
    out_d = nc.dram_tensor("vout", [128, 2 * OC], dt.float32,
                           kind="ExternalOutput").ap()

    # ---- static SBUF ----
    def sb(name, shape, dtype):
        return nc.alloc_sbuf_tensor(name, list(shape), dtype).ap()

    u16 = sb("u16_sb", [128, OC * H * NI], dt.float16)       # 46KB/part
    uT = sb("uT_sb", [128, H * OC * NC9 * 128], dt.float16)  # 46KB/part
    f32pack_sb = sb("f32pack_sb", [128, 338], dt.float32)
    f16pack_sb = sb("f16pack_sb", [128, 208], dt.float16)
    base8_sb = f32pack_sb[:, 0:8]
    b82a3_sb = f32pack_sb[:, 8:248]
    jmask_sb = f32pack_sb[:80, 248:258]
    ident80_sb = f32pack_sb[:80, 258:338]
    b82a16_sb = f16pack_sb[:, 0:80]
    b82aT16_sb = f16pack_sb[:80, 80:208]
    bb = [sb(f"bbsb{h}", [OC * BLOC, NI], dt.float32) for h in range(H)]
    # sfat_all[h]: 10 W1 stationaries packed at 88-col stride; the j-th
    # stationary view is cols [80j, 80j+80) whose nonzero 8-col block at
    # 88j lands at stationary column 8j -> out rows (j,b), as required.
    sfat_all = [sb(f"sfat_all{h}", [128, 800], dt.float16) for h in range(H)]
    f80 = [sb(f"f80_{h}", [OC * BLOC, 1], dt.float32) for h in range(H)]
    s_sb = sb("s_sb", [128, H * OC], dt.float32)
    # mini-squash scratch (per h), all in [80, *] layout
    ssqh = [sb(f"ssqh{h}", [128, OC], dt.float16) for h in range(H)]
    jscr = [sb(f"jscr{h}", [OC * BLOC, OC], dt.float32) for h in range(H)]
    sqe80 = [sb(f"sqe80_{h}", [OC * BLOC, 1], dt.float32) for h in range(H)]
    lnx80 = [sb(f"lnx80_{h}", [OC * BLOC, 1], dt.float32) for h in range(H)]
    r80 = [sb(f"r80_{h}", [OC * BLOC, 1], dt.float32) for h in range(H)]
    den80 = [sb(f"den80_{h}", [OC * BLOC, 1], dt.float32) for h in range(H)]
    rec80 = [sb(f"rec80_{h}", [OC * BLOC, 1], dt.float32) for h in range(H)]
    # W4 extraction scratch
    mskd = [sb(f"mskd{h}", [128, 3 * OC * BLOC], dt.float32) for h in range(H)]
    # final squash scratch
    fj16 = [sb(f"fj16_{h}", [OC * BLOC, OC], dt.float16) for h in range(H)]
    v_sb = [sb(f"v_sb{h}", [128, OC], dt.float32) for h in range(H)]

    def uview(j, h):
        off = (j * H + h) * NI
        return u16[:, off:off + NI]

    def uTview(h, j, ci):
        off = ((h * OC + j) * NC9 + ci) * 128
        return uT[:, off:off + 128]

    with tile.TileContext(nc) as tc:
        from contextlib import ExitStack
        with ExitStack() as ctx:
            psA = ctx.enter_context(
                tc.tile_pool(name="psA", bufs=int(os.environ.get("K_PSA", "3")), space="PSUM"))
            psB = ctx.enter_context(
                tc.tile_pool(name="psB", bufs=int(os.environ.get("K_PSB", "2")), space="PSUM"))
            psS = ctx.enter_context(
                tc.tile_pool(name="psS", bufs=2, space="PSUM"))
            sc = ctx.enter_context(
                tc.tile_pool(name="sc", bufs=int(os.environ.get("K_SCBUFS", "3"))))
            ec = ctx.enter_context(
                tc.tile_pool(name="ec", bufs=int(os.environ.get("K_ECBUFS", "3"))))

            for _rep in range(TIME_REPS):
                # ---- loads, ordered to unblock the h0 stream first:
                # u16[h0], consts, u16[h1], uT[h0], uT[h1] ----
                for j in range(OC):
                    off = (j * H + 0) * NI
                    nc.sync.dma_start(u16[:, off:off + NI], u16_d[:, j, 0, :])
                nc.sync.dma_start(f32pack_sb[:], f32pack_d)
                nc.sync.dma_start(f16pack_sb[:], f16pack_d)
                for j in range(OC):
                    off = (j * H + 1) * NI
                    nc.sync.dma_start(u16[:, off:off + NI], u16_d[:, j, 1, :])
                for h in range(H):
                    off = h * OC * NC9 * 128
                    nc.sync.dma_start(
                        uT[:, off:off + OC * NC9 * 128], uT_d[:, h, :, :])

                # ---- init bb and sfat ----
                for h in range(H):
                    if general_b:
                        nc.sync.dma_start(bb[h][:], bb0_d)
                    else:
                        nc.gpsimd.memset(bb[h][:], 0.0)
                for h in range(H):
                    nc.gpsimd.memset(sfat_all[h][:], 0.0)

                def build_sfat_all(h):
                    # all 10 blocks in one strided TT:
                    # sfat_all[p, 88j + b'] = b82a[p, (j,b')] * s[p, (j,h)]
                    nc.vector.tensor_tensor(
                        sfat_all[h][:].rearrange(
                            "p (a x) -> p a x", x=BLOC)[:, ::11, :],
                        b82a3_sb[:, :OC * BLOC].rearrange(
                            "p (j x) -> p j x", j=OC),
                        s_sb[:, h::2][:, :, None].broadcast_to(
                            [128, OC, BLOC]),
                        op=ALU.mult)

                def mini_squash(h):
                    """f80[h] <- squash scale, computed in [80,*] layout.

                    f = sq/((1+sq)*sqrt(sq+EPS)); sqe = sq+EPS stands in for
                    sq (EPS=1e-8 absolute, negligible).
                    """
                    s_h = s_sb[:, h::2]  # [128, OC] strided view
                    nc.vector.tensor_tensor(ssqh[h][:], s_h, s_h, op=ALU.mult)
                    sq_ps = psB.tile([OC * BLOC, OC], dt.float32, tag="bank",
                                     name="sq80_ps")
                    nc.tensor.matmul(sq_ps[:], b82a16_sb[:], ssqh[h][:],
                                     start=True, stop=True)
                    nc.vector.scalar_tensor_tensor(
                        out=jscr[h][:], in0=sq_ps[:], scalar=1.0,
                        in1=jmask_sb[:], op0=ALU.mult, op1=ALU.mult,
                        accum_out=sqe80[h][:])
                    nc.vector.tensor_scalar_add(sqe80[h][:], sqe80[h][:], EPS)
                    nc.scalar.activation(lnx80[h][:], sqe80[h][:], AF.Ln)
                    nc.scalar.activation(r80[h][:], lnx80[h][:], AF.Exp,
                                         scale=0.5)
                    # den = (sqe + 1) * r
                    nc.vector.tensor_scalar_add(den80[h][:], sqe80[h][:], 1.0)
                    nc.vector.tensor_scalar(
                        out=den80[h][:], in0=den80[h][:],
                        scalar1=r80[h][:, 0:1], scalar2=None, op0=ALU.mult)
                    nc.vector.reciprocal(rec80[h][:], den80[h][:])
                    nc.vector.tensor_scalar(
                        out=f80[h][:], in0=sqe80[h][:],
                        scalar1=rec80[h][:, 0:1], scalar2=None, op0=ALU.mult)

                # ---- init s0 for one h-stream (called per phase) ----
                if general_b:
                    c0_sb = sc.tile([128, OC * NI], dt.float16, tag="c0",
                                    name="c0_sb", bufs=1)
                    nc.sync.dma_start(c0_sb[:], c0_d)

                def emit_init(h):
                    if general_b:
                        for j in range(OC):
                            col = 2 * j + h
                            scr = sc.tile([128, NI], dt.float16, tag="scr",
                                          name="scr")
                            nc.vector.scalar_tensor_tensor(
                                out=scr[:], in0=uview(j, h), scalar=1.0,
                                in1=c0_sb[:, j * NI:(j + 1) * NI],
                                op0=ALU.mult, op1=ALU.mult,
                                accum_out=s_sb[:, col:col + 1])
                    else:
                        for j in range(OC):
                            col = 2 * j + h
                            if j % 2 == 0:
                                nc.vector.reduce_sum(
                                    s_sb[:, col:col + 1], uview(j, h),
                                    axis=AX.X)
                            else:
                                scr = sc.tile([128, NI], dt.float16,
                                              tag="scr", name="scr")
                                nc.scalar.activation(
                                    scr[:], uview(j, h), AF.Identity,
                                    accum_out=s_sb[:, col:col + 1])
                        nc.vector.tensor_scalar_mul(
                            s_sb[:, h::2], s_sb[:, h::2], 1.0 / OC)
                    build_sfat_all(h)
                    mini_squash(h)

                # ---- routing iterations ----
                # Two phase-shifted h-streams: emit W1+softmax for phase
                # (it, h), then the PREVIOUS phase's W4+tail, so each
                # stream's cross-engine tail latency hides under the other
                # stream's W1 matmuls. Pipeline pieces: unit c covers i-cols
                # CHUNKS[c] = transpose-chunks TCH[c].
                TCH = [(0, 4), (4, 8), (8, 9)]
                NC = len(CHUNKS)
                bbT_t, eT, zrT, cT16, s_ps_t = {}, {}, {}, {}, {}

                def emit_head(it, h, w4_mid=None):
                    """W1 (with the previous phase's W4 matmuls mid-stream
                    and bb-transposes at the end) + softmax."""
                    eT[(it, h)] = ec.tile([128, NC9 * 80], dt.float32,
                                          tag="eT", name="eT")
                    zrT[(it, h)] = ec.tile([128, NC9 * BLOC], dt.float32,
                                           tag="zrT", name="zrT")
                    cT16[(it, h)] = ec.tile([128, NC9 * 80], dt.float16,
                                            tag="ct", name="cT16")

                    def emit_w1(ci):
                        c0, c1 = CHUNKS[ci]
                        a_c = psB.tile([OC * BLOC, 512], dt.float32,
                                       tag="bank", name="a_c")
                        for j in range(OC):
                            nc.tensor.matmul(
                                a_c[:, :c1 - c0],
                                sfat_all[h][:, 80 * j:80 * (j + 1)],
                                uview(j, h)[:, c0:c1],
                                start=(j == 0), stop=(j == OC - 1))
                        # GPSIMD cannot access PSUM (a_c), so DVE only here
                        nc.vector.scalar_tensor_tensor(
                            out=bb[h][:, c0:c1], in0=a_c[:, :c1 - c0],
                            scalar=f80[h][:, 0:1], in1=bb[h][:, c0:c1],
                            op0=ALU.mult, op1=ALU.add)

                    def emit_transp(c):
                        t0, t1 = TCH[c]
                        bbT = psA.tile([128, 320], dt.float32,
                                       tag="bbT", name="bbT")
                        for ci in range(t0, t1):
                            nc.tensor.transpose(
                                bbT[:, (ci - t0) * 80:(ci - t0 + 1) * 80],
                                bb[h][:, ci * 128:(ci + 1) * 128],
                                ident80_sb[:])
                        bbT_t[(it, h, c)] = bbT

                    emit_w1(0)
                    emit_w1(1)
                    if w4_mid is not None:
                        emit_w4(*w4_mid)
                    emit_w1(2)
                    emit_transp(0)
                    emit_transp(1)
                    emit_transp(2)
                    for c in range(NC):
                        t0, t1 = TCH[c]
                        nc.scalar.activation(
                            eT[(it, h)][:, t0 * 80:t1 * 80],
                            bbT_t[(it, h, c)][:, :(t1 - t0) * 80], AF.Exp)

                def emit_head_b(it, h):
                    """Z-reduce, reciprocal, divide (kept after the previous
                    phase's extract so that chain isn't queued behind these
                    on DVE)."""
                    final_phase = (it == NITER - 1 and h == 1)
                    for c in range(NC):
                        t0, t1 = TCH[c]
                        ncc = t1 - t0
                        nc.vector.reduce_sum(
                            zrT[(it, h)][:, t0 * BLOC:t1 * BLOC].rearrange(
                                "p (ci b) -> p ci b", ci=ncc),
                            eT[(it, h)][:, t0 * 80:t1 * 80].rearrange(
                                "p (ci j b) -> p ci b j", ci=ncc, j=OC),
                            axis=AX.X)
                        if not (c == 2 or final_phase):
                            # GpSimd has no divide: pre-invert its pieces
                            nc.vector.reciprocal(
                                zrT[(it, h)][:, t0 * BLOC:t1 * BLOC],
                                zrT[(it, h)][:, t0 * BLOC:t1 * BLOC])

                    for c in range(NC):
                        t0, t1 = TCH[c]
                        ncc = t1 - t0
                        # big pieces on GpSimd (multiply by 1/Z), the last
                        # piece and the whole final phase on DVE via a
                        # direct divide (one less chain stage)
                        use_dve = c == 2 or final_phase
                        eng = nc.vector if use_dve else nc.gpsimd
                        eng.tensor_tensor(
                            cT16[(it, h)][:, t0 * 80:t1 * 80].rearrange(
                                "p (ci j b) -> p ci j b", ci=ncc, j=OC),
                            eT[(it, h)][:, t0 * 80:t1 * 80].rearrange(
                                "p (ci j b) -> p ci j b", ci=ncc, j=OC),
                            zrT[(it, h)][:, t0 * BLOC:t1 * BLOC].rearrange(
                                "p (ci b) -> p ci b", ci=ncc)[:, :, None, :]
                            .broadcast_to([128, ncc, OC, BLOC]),
                            op=ALU.divide if use_dve else ALU.mult)

                def emit_w4(it, h):
                    """W4 weighted-sum matmuls; each (j, piece) accumulation
                    group opens and closes within its piece (one open group
                    per psum bank); piece partials land at (j, c, b')."""
                    s_ps = psS.tile([128, 3 * OC * BLOC], dt.float32,
                                    tag="sps", name="s_ps")
                    s_ps_t[(it, h)] = s_ps
                    for c in range(NC):
                        t0, t1 = TCH[c]
                        for j in range(OC):
                            col = (j * 3 + c) * BLOC
                            for ci in range(t0, t1):
                                nc.tensor.matmul(
                                    s_ps[:, col:col + BLOC],
                                    uTview(h, j, ci),
                                    cT16[(it, h)][:, ci * 80 + j * BLOC:
                                                  ci * 80 + (j + 1) * BLOC],
                                    start=(ci == t0), stop=(ci == t1 - 1))

                def emit_extract(it, h):
                    """Diagonal extract, sfat/squash (+final v on the last
                    iteration)."""
                    last = it == NITER - 1
                    s_ps = s_ps_t[(it, h)]
                    nc.vector.tensor_tensor(mskd[h][:], s_ps[:],
                                            b82a3_sb[:], op=ALU.mult)
                    nc.vector.reduce_sum(
                        s_sb[:, h::2],
                        mskd[h][:].rearrange("p (j cb) -> p j cb", j=OC),
                        axis=AX.X)
                    if not last:
                        build_sfat_all(h)
                    mini_squash(h)
                    if last:
                        # final: v = f * s, with f replicated from [80,1]
                        # to [(b,d), j] via jmask scale + b82aT matmul
                        nc.vector.tensor_scalar(
                            out=fj16[h][:], in0=jmask_sb[:],
                            scalar1=f80[h][:, 0:1], scalar2=None,
                            op0=ALU.mult)
                        f_ps = psB.tile([128, OC], dt.float32,
                                        tag="bank", name="f_ps")
                        nc.tensor.matmul(f_ps[:], b82aT16_sb[:],
                                         fj16[h][:], start=True, stop=True)
                        nc.vector.tensor_tensor(
                            v_sb[h][:], s_sb[:, h::2], f_ps[:], op=ALU.mult)
                        nc.sync.dma_start(
                            out_d[:, h * OC:(h + 1) * OC], v_sb[h][:])

                pending = None
                for it in range(NITER):
                    for h in range(H):
                        if it == 0:
                            emit_init(h)
                        # iteration-0 W4s wait on the uT upload: emit them at
                        # phase end so they don't block this phase's W1 and
                        # transposes in the in-order PE stream.
                        # only (0,h0)'s W4 must defer to phase end (its
                        # uT piece is still in flight mid-W1)
                        mid_ok = pending is not None and pending != (0, 0)
                        emit_head(it, h,
                                  w4_mid=pending if mid_ok else None)
                        if pending is not None and mid_ok:
                            emit_extract(*pending)
                        emit_head_b(it, h)
                        if pending is not None and not mid_ok:
                            emit_w4(*pending)
                            emit_extract(*pending)
                        pending = (it, h)
                emit_w4(*pending)
                emit_extract(*pending)

    nc.compile()
    return nc


def _get_program(general_b):
    key = bool(general_b)
    if key not in _PROG_CACHE:
        _PROG_CACHE[key] = _build_program(key)
    return _PROG_CACHE[key]


def _prep_inputs(u_predict, b):
    """Host-side shard + layout transform. Returns (in_maps, general_b)."""
    general_b = bool(np.any(b != 0.0))
    consts = _build_consts()
    u16 = u_predict.astype(np.float16)
    u6 = u16.reshape(NCORES, H, BLOC, IC, OC, D)
    ut = np.ascontiguousarray(u6.transpose(0, 2, 5, 4, 1, 3))
    ut = ut.reshape(NCORES, 128, OC, H, NI)
    # uT[c, i_lo, h, j, ci*128 + p] = ut[c, p, j, h, ci*128 + i_lo]
    u5 = ut.reshape(NCORES, 128, OC, H, NC9, 128)
    uTt = np.ascontiguousarray(u5.transpose(0, 5, 3, 2, 4, 1))
    uTt = uTt.reshape(NCORES, 128, H, OC, NC9 * 128)

    extra = {}
    if general_b:
        bm = b.astype(np.float64)
        e = np.exp(bm - bm.max(axis=1, keepdims=True))
        c0 = (e / e.sum(axis=1, keepdims=True)).astype(np.float16)  # [IC, OC]
        c0rep = np.ascontiguousarray(
            np.broadcast_to(c0.T[None, :, :], (128, OC, NI))).astype(
                np.float16)
        bt = b.astype(np.float32).T  # [OC, NI]
        bb0 = np.ascontiguousarray(
            np.repeat(bt[:, None, :], BLOC, axis=1)).reshape(OC * BLOC, NI)
        extra = {"c0rep": c0rep, "bb0": bb0}

    in_maps = []
    for c in range(NCORES):
        m = {"u16": ut[c], "uT": uTt[c]}
        m.update(consts)
        m.update(extra)
        in_maps.append(m)
    return in_maps, general_b


def _gather_output(results):
    out = np.empty((B, OC, D), np.float32)
    for c in range(NCORES):
        v = results[c]["vout"]                  # [p=(bl,d), col=(h*OC+j)]
        v4 = v.reshape(BLOC, D, H, OC)          # bl, d, h, j
        out[c * BL:(c + 1) * BL] = v4.transpose(2, 0, 3, 1).reshape(
            BL, OC, D)
    return out


def kernel(u_predict, b=None, **kw):
    u_predict = np.asarray(u_predict, dtype=np.float32)
    if b is None:
        b = np.zeros((IC, OC), np.float32)
    b = np.asarray(b, dtype=np.float32)
    in_maps, general_b = _prep_inputs(u_predict, b)
    nc = _get_program(general_b)

    if os.environ.get("BASS_KERNEL_SIM"):
        from concourse.bass_interp import CoreSim
        sim = CoreSim(nc, trace=False)
        for name, arr in in_maps[0].items():
            sim.tensor(name)[:] = arr
        sim.simulate(check_with_hw=False)
        v0 = np.array(sim.tensor("vout"))
        out = np.empty((B, OC, D), np.float32)
        v4 = v0.reshape(BLOC, D, H, OC)
        out[:BL] = v4.transpose(2, 0, 3, 1).reshape(BL, OC, D)
        return out  # NOTE: only core 0 valid in sim mode

    from concourse import bass_utils
    trace = bool(os.environ.get("BASS_KERNEL_TRACE"))
    res = bass_utils.run_bass_kernel_spmd(
        nc, in_maps, core_ids=list(range(NCORES)), trace=trace)
    kernel.last_results = res
    return _gather_output(res.results)
